# revision 64
# baseline (speedup 1.0000x reference)
"""AttentionSequencePoolingLayer (DIN attention) on 8 trn2 NeuronCores.

Data-parallel over batch: B=2048 -> BL=256 per core.

Math per (b,t):  att = concat([q,k,q-k,q*k]) @ W1 + b1
  Split W1 = [W1a;W1b;W1c;W1d] by the four concat blocks:
    att = k @ (W1b-W1c + diag(q_b)W1d) + (q_b@(W1a+W1c) + b1)
        = k @ Wf_b + U_b
  Host folds the per-batch first layer into WfU_b = [Wf_b; U_b] [65,80]
  and sends kin_b = [k_b; ones] [65,200] so L1 is ONE fp16 matmul per
  batch with the bias included (no per-batch bias activations).

  h1 = sigmoid(att)                  [80, T]   (feature-major)
  h2 = sigmoid(h1 @ W2 + b2)         token-major [T, 40] out of PE
       (b2 folded via a ones-row in h1: lhsT=[h1;1] [81,100], rhs=[W2;b2])
  Fused scoring+pooling: instead of s = h2@W3+b3 then out = s_masked @ K,
    M_b   = sum_t [h2_t; 1] (x) kmask_t   [41, 64]   (PE, 2 half matmuls)
    out_b = M_b^T @ [W3; b3]              [64]       (PE)
  which equals sum_t mask_t (h2_t.W3 + b3) k_t exactly.

All PE inputs fp16 (1 cycle/row like bf16 but ~8x better mantissa; fp32
would be 4 cycles/row), PSUM accumulation fp32; M/final stage kept fp32.
Sigmoids run on the scalar engine over multi-bank PSUM chunks (8 batches
per instruction) to amortize the ~185ns per-instruction access latency.
The emission is software-pipelined (L2 one group behind h1, M/final three
behind) so the in-order Act/PE queues never block on younger stages.
"""
import numpy as np

import concourse.bacc as bacc
import concourse.bass as bass
import concourse.mybir as mybir
import concourse.tile as tile
from concourse.bass_utils import run_bass_kernel_spmd

B, T, E = 2048, 200, 64
H1, H2 = 80, 40
NCORES = 8
BL = B // NCORES          # 256 batches per core
TH = T // 2               # 100-token halves (PE contraction <= 128)
BT = 32                   # batches per DMA tile
G = 8                     # batches per compute group
NG = BL // G              # 32 groups

F16 = mybir.dt.float16
F32 = mybir.dt.float32

_cache = {}
TRACE = False          # set True (e.g. from test.py) to profile; fills LAST
LAST = {"exec_time_ns": None, "result": None}


def _build(tgs):
    nc = bacc.Bacc(None, target_bir_lowering=False)

    ones_d = nc.dram_tensor("ones", [1, 4, 2 * T], F16, kind="ExternalInput")
    kin_d = nc.dram_tensor("kin", [E + 1, BL, T], F16, kind="ExternalInput")
    ktok_d = nc.dram_tensor("ktok", [TH, BL, 2, E], F16, kind="ExternalInput")
    wfu_d = nc.dram_tensor("wfu", [E + 1, BL, H1], F16, kind="ExternalInput")
    w2b_d = nc.dram_tensor("w2b", [H1 + 1, H2], F16, kind="ExternalInput")
    w3c_d = nc.dram_tensor("w3c", [H2 + 1, 1], F32, kind="ExternalInput")
    out_d = nc.dram_tensor("out", [BL, E], F16, kind="ExternalOutput")

    SIG = mybir.ActivationFunctionType.Sigmoid

    with tile.TileContext(nc) as tc:
        with (
            tc.tile_pool(name="io", bufs=6) as io,
            tc.tile_pool(name="const", bufs=1) as const,
            tc.tile_pool(name="sb", bufs=2) as sbp,
            tc.tile_pool(name="ph1", bufs=2, space="PSUM") as ph1,
            tc.tile_pool(name="ph2", bufs=1, space="PSUM") as ph2,
            tc.tile_pool(name="pm", bufs=1, space="PSUM") as pmp,
            tc.tile_pool(name="po", bufs=1, space="PSUM") as pop,
        ):
            GPB = BT // G          # groups per DMA block
            io_tiles = {}

            # Trigger the sigmoid table load on the scalar engine at t=0 so
            # the first real activation doesn't pay the ~1.3us load; the
            # tiny memset satisfies its input dependency immediately.
            warm = const.tile([1, 512], F16)
            nc.vector.memset(warm[:], 0.0)
            nc.scalar.activation(warm[0:1, 0:1], warm[0:1, 0:1], SIG)

            def load_block(bt, ktok_on_act=False):
                b0 = bt * BT
                kin_t = io.tile([E + 1, BT, T], F16, tag="kin", name="kin_t")
                ktok_t = io.tile([TH, BT, 2, E], F16, tag="ktok", name="ktok_t")
                wfu_t = io.tile([E + 1, BT, H1], F16, tag="wfu", name="wfu_t")
                nc.sync.dma_start(kin_t[:], kin_d[:, b0 : b0 + BT, :])
                nc.sync.dma_start(wfu_t[:], wfu_d[:, b0 : b0 + BT, :])
                eng = nc.scalar if ktok_on_act else nc.sync
                eng.dma_start(ktok_t[:], ktok_d[:, b0 : b0 + BT, :, :])
                io_tiles[bt] = (kin_t, ktok_t, wfu_t)

            # block-0 is loaded first-group-first so compute starts after
            # ~0.7 MB of DMA instead of 4 MB; the remainder follows with
            # ktok ahead of kin/wfu to meet the M-stage's earlier deadline.
            def load_block0():
                kin_t = io.tile([E + 1, BT, T], F16, tag="kin", name="kin_t")
                ktok_t = io.tile([TH, BT, 2, E], F16, tag="ktok", name="ktok_t")
                wfu_t = io.tile([E + 1, BT, H1], F16, tag="wfu", name="wfu_t")
                nc.sync.dma_start(kin_t[:, 0:G, :], kin_d[:, 0:G, :])
                nc.sync.dma_start(wfu_t[:, 0:G, :], wfu_d[:, 0:G, :])
                nc.sync.dma_start(ktok_t[:, 0:G, :, :], ktok_d[:, 0:G, :, :])
                # the remainder is issued from the (idle) Act sequencer so
                # the SP queue's issue serialization doesn't starve ramp-in
                nc.scalar.dma_start(ktok_t[:, G:BT, :, :], ktok_d[:, G:BT, :, :])
                nc.scalar.dma_start(kin_t[:, G:BT, :], kin_d[:, G:BT, :])
                nc.scalar.dma_start(wfu_t[:, G:BT, :], wfu_d[:, G:BT, :])
                io_tiles[0] = (kin_t, ktok_t, wfu_t)

            load_block0()

            # h1 feature-major SBUF tiles, row 80 = ones (folds b2 into L2);
            # engines can't address partition base 80 (mod-32 rule) so the
            # ones row is DMAed in from DRAM. Buffer 0's ones row is needed
            # by L2(0) early, so it's DMAed before block 1's inputs.
            h1sb = [
                const.tile([H1 + 1, 4, 2 * T], F16, name=f"h1sb{i}")
                for i in range(3)
            ]
            nc.sync.dma_start(h1sb[0][H1 : H1 + 1, :, :], ones_d[:])
            nc.sync.dma_start(h1sb[1][H1 : H1 + 1, :, :], ones_d[:])
            nc.sync.dma_start(h1sb[2][H1 : H1 + 1, :, :], ones_d[:])
            load_block(1)
            w2b_s = const.tile([H1 + 1, H2], F16)
            w3c_s = const.tile([H2 + 1, 1], F32)
            outsb = const.tile([E, BL], F16)
            nc.sync.dma_start(w2b_s[:], w2b_d[:])
            nc.sync.dma_start(w3c_s[:], w3c_d[:])
            for bt in range(2, BL // BT):
                load_block(bt)
            # h2 token-major SBUF tiles, col 40 of each slot = ones (colsum)
            h2sb = []
            for i in range(4):
                t_ = const.tile([TH, 2 * G, H2 + 1], F16, name=f"h2sb{i}")
                nc.vector.memset(t_[:], 0.0)
                nc.vector.memset(t_[:, :, H2 : H2 + 1], 1.0)
                h2sb.append(t_)

            # Warm the PE p-state with throwaway matmuls while the first
            # input DMAs are in flight (PE ramps 0.65->2.4 GHz after ~3us
            # of continuous execution).
            wps = pop.tile([128, 128], F32, tag="ops", name="wps")
            for _ in range(5):
                nc.tensor.matmul(
                    wps[0:1, 0:128], warm[:, 0:1], warm[:, 0:128],
                    start=True, stop=True,
                )

            # Units: 8-batch groups while Tg > 100 (two token halves),
            # merged into 16-batch wide units once Tg <= 100 (one half,
            # 4 batches per PSUM bank) — halving the per-unit pipeline
            # latency in the short-group tail.
            units = []
            g = 0
            while g < NG:
                if tgs[g] <= TH and g + 1 < NG:
                    units.append((g, 2))
                    g += 2
                else:
                    units.append((g, 1))
                    g += 1
            nun = len(units)
            # cumulative batch positions covered after each unit
            cum = []
            c = 0
            for g0, w in units:
                c += 8 * w
                cum.append(c)

            def stage_l1(u):
                # L1: att+U -> psum + sigmoid in two double-buffered
                # 2-bank halves (2 batches/bank narrow, 4 wide).
                g0, w = units[u]
                tg = tgs[g0]
                h1s = h1sb[u % 3]
                bpb = 2 * w            # batches per psum bank
                for half in range(2):
                    h1ps = ph1.tile([128, 2, 512], F32, tag="h1ps", name="h1ps")
                    for jj in range(2 * bpb):
                        pos = 8 * g0 + half * 2 * bpb + jj
                        kin_t, _, wfu_t = io_tiles[pos // BT]
                        nc.tensor.matmul(
                            h1ps[0:H1, jj // bpb, (jj % bpb) * tg : (jj % bpb) * tg + tg],
                            wfu_t[:, pos % BT, :],
                            kin_t[:, pos % BT, 0:tg],
                            start=True,
                            stop=True,
                        )
                    nc.scalar.activation(
                        h1s[0:H1, 2 * half : 2 * half + 2, 0 : bpb * tg],
                        h1ps[0:H1, :, 0 : bpb * tg],
                        SIG,
                    )

            def stage_l2(u):
                # L2: h2 token-major -> psum slots + sigmoid. Token halves
                # split at 100 (the ktok layout boundary).
                g0, w = units[u]
                tg = tgs[g0]
                nh = 1 if tg <= TH else 2
                bpb = 2 * w
                h1s = h1sb[u % 3]
                h2s = h2sb[u % 4]
                h2ps = ph2.tile([TH, 2 * G, 64], F32, tag="h2ps", name="h2ps")
                for jj in range(8 * w):
                    bank = 2 * (jj // (2 * bpb)) + (jj % (2 * bpb)) // bpb
                    off = (jj % bpb) * tg
                    for h in range(nh):
                        hl = min(tg, TH) if h == 0 else tg - TH
                        nc.tensor.matmul(
                            h2ps[0:hl, nh * jj + h, 0:H2],
                            h1s[:, bank, off + h * TH : off + h * TH + hl],
                            w2b_s[:],
                            start=True,
                            stop=True,
                        )
                nc.scalar.activation(
                    h2s[:, 0 : 8 * w * nh, 0:H2],
                    h2ps[:, 0 : 8 * w * nh, 0:H2],
                    SIG,
                )

            def stage_m(u):
                # M: [41, 64] per batch = sum_t [h2_t;1] (x) kmask_t, then
                # out_b [64,1] = M_b^T @ [W3;b3]. Wide units run two
                # 8-batch rounds through the single mps bank.
                g0, w = units[u]
                tg = tgs[g0]
                nh = 1 if tg <= TH else 2
                h2s = h2sb[u % 4]
                for r in range(w):
                    p0 = 8 * g0 + r * G
                    mps = pmp.tile([H2 + 1, G, E], F32, tag="mps", name="mps")
                    for j in range(G):
                        pos = p0 + j
                        _, ktok_t, _ = io_tiles[pos // BT]
                        for h in range(nh):
                            nc.tensor.matmul(
                                mps[:, j, :],
                                h2s[:, nh * (r * G + j) + h, :],
                                ktok_t[:, pos % BT, h, :],
                                start=(h == 0),
                                stop=(h == nh - 1),
                            )
                    msb = sbp.tile([H2 + 1, G * E], F32, tag="msb", name="msb")
                    nc.vector.tensor_scalar_add(msb[:], mps[:, :, :], 0.0)

                    ob = G * ((u + r) % 2)
                    ops = pop.tile([128, 128], F32, tag="ops", name="ops")
                    for j in range(G):
                        nc.tensor.matmul(
                            ops[0:E, ob + j : ob + j + 1],
                            msb[:, j * E : (j + 1) * E],
                            w3c_s[:],
                            start=True,
                            stop=True,
                        )
                    nc.vector.tensor_scalar_add(
                        outsb[:, p0 : p0 + G], ops[0:E, ob : ob + G], 0.0
                    )

            # Software-pipelined emission over units: while h1-sigmoid of
            # unit u runs on Act, PE does L2 of u-1 and M/final of u-3 —
            # all engine queues are in-order, so older work is emitted
            # first. Output ships in three pieces to shorten the drain.
            mid_u = cum.index(128) if 128 in cum else None
            p2 = max((c for c in cum if c <= 240), default=None)
            fin = const.tile([128, 2, E], F16)
            for i in range(nun + 3):
                if i < nun:
                    stage_l1(i)
                if 1 <= i < nun + 1:
                    stage_l2(i - 1)
                if i >= 3:
                    stage_m(i - 3)
                    if mid_u is not None and i - 3 == mid_u:
                        # first 128 batches done: ship while the rest runs
                        nc.sync.dma_start_transpose(
                            fin[:, 0, :], outsb[:, 0:128]
                        )
                        nc.sync.dma_start(out_d[0:128, :], fin[:, 0, :])
                    if p2 is not None and cum[i - 3] == p2 and p2 > 128:
                        # batches 128..p2 done: overlapped strided ship
                        nc.sync.dma_start(
                            out_d[128:p2, :].rearrange("a b -> b a"),
                            outsb[:, 128:p2],
                        )

            # remaining tail batches: small element-strided transposing
            # DMA beats the XBAR+copy+DMA chain latency for this size
            pt = p2 if (p2 is not None and p2 > 128) else 128
            if pt < BL:
                nc.sync.dma_start(
                    out_d[pt:BL, :].rearrange("a b -> b a"), outsb[:, pt:BL]
                )

    nc.compile()
    return nc


def kernel(query, keys, keys_length, W1, b1, W2, b2, W3, b3):
    query = np.asarray(query, np.float32)
    keys = np.asarray(keys, np.float32)
    keys_length = np.asarray(keys_length, np.int32)
    W1 = np.asarray(W1, np.float32); b1 = np.asarray(b1, np.float32)
    W2 = np.asarray(W2, np.float32); b2 = np.asarray(b2, np.float32)
    W3 = np.asarray(W3, np.float32); b3 = np.asarray(b3, np.float32)
    bf = np.float16

    A = W1[0:E] + W1[2 * E : 3 * E]          # q coeff
    Bw = W1[E : 2 * E] - W1[2 * E : 3 * E]   # k coeff
    C = W1[3 * E : 4 * E]                    # q*k coeff

    q2 = query[:, 0, :]                      # [B, E]
    U = q2 @ A + b1                          # [B, H1]
    # folded per-batch first-layer weight + bias row: [B, 65, 80]
    WfU = np.empty((B, E + 1, H1), np.float32)
    WfU[:, :E] = Bw[None] + q2[:, :, None] * C[None]
    WfU[:, E] = U

    # [65, B, T]: keys feature-major with a ones row
    kin = np.empty((E + 1, B, T), np.float32)
    kin[:E] = keys.transpose(2, 0, 1)
    kin[E] = 1.0

    mask = (np.arange(T)[None, :] < keys_length).astype(np.float32)  # [B, T]
    kmask = keys * mask[:, :, None]          # [B, T, E]
    ktok = kmask.reshape(B, 2, TH, E).transpose(2, 0, 1, 3)  # [TH, B, 2, E]

    w2b = np.concatenate([W2, b2.reshape(1, H2)], 0)         # [81, 40]
    w3c = np.concatenate([W3.reshape(H2, 1), b3.reshape(1, 1)], 0)  # [41, 1]

    # Sort batches by length (descending) and deal them stratified across
    # cores: length-rank slice i holds sorted[64i:64i+64], core c gets 8 of
    # those. Every core sees the same length profile (perfect balance) and
    # the device program is specialized to each group's max length tgs[g].
    # Long groups run first; the short tail's input blocks prefetch during
    # the long phase (total DMA time < total compute time), so the tail is
    # never DMA-starved.
    lens = np.clip(keys_length.reshape(B), 1, T).astype(np.int64)
    order = np.argsort(-lens, kind="stable")
    tgs = tuple(int(lens[order[64 * g]]) for g in range(NG))
    key = tgs
    perm = [np.concatenate([order[64 * g + 8 * c : 64 * g + 8 * c + 8]
                            for g in range(NG)]) for c in range(NCORES)]

    if key not in _cache:
        _cache[key] = _build(tgs)
    nc = _cache[key]

    kin16 = kin.astype(bf)
    ktok16 = ktok.astype(bf)
    wfu16 = WfU.transpose(1, 0, 2).astype(bf)
    w2b16 = w2b.astype(bf)
    w3c16 = w3c.astype(np.float32)

    in_maps = []
    for c in range(NCORES):
        p = perm[c]
        in_maps.append({
            "ones": np.ones((1, 4, 2 * T), bf),
            "kin": np.ascontiguousarray(kin16[:, p, :]),
            "ktok": np.ascontiguousarray(ktok16[:, p, :, :]),
            "wfu": np.ascontiguousarray(wfu16[:, p, :]),
            "w2b": w2b16,
            "w3c": w3c16,
        })

    res = run_bass_kernel_spmd(nc, in_maps, list(range(NCORES)), trace=TRACE)
    if TRACE:
        LAST["exec_time_ns"] = res.exec_time_ns
        LAST["result"] = res
    out = np.empty((B, E), np.float32)
    for c in range(NCORES):
        out[perm[c]] = np.asarray(res.results[c]["out"]).astype(np.float32)
    return out.reshape(B, 1, E)
